# revision 1
# baseline (speedup 1.0000x reference)
"""Bass/Tile TRN2 kernel for nn_LAN_4320737100678 (dense transformer block).

Data-parallel over the batch axis across 8 NeuronCores (4 batches/core).
All activations are kept feature-major ([E, L] per batch) so that every
BatchNorm reduction and the softmax run along the free axis, and the BN
affine+ELU applications are single activation-engine passes with
per-partition scale/bias. The five BatchNorm moment sets are globalized
with four tiny in-kernel AllReduces (BN2+BN3 share one round).

The sliding-window "unfold" (W=5) is never materialized: stage 1 is
computed as 5 shifted matmuls accumulating into PSUM from a zero-padded
copy of m1^T.
"""

import os
import sys

sys.path.insert(0, "/opt/trn_rl_repo")

import numpy as np

import concourse.bass as bass
import concourse.tile as tile
from concourse import mybir
from concourse.bass_utils import run_bass_kernel_spmd
from concourse.masks import make_identity
from concourse.vector_clock import ScopedClock

N_CORES = 8
B, L, E, W = 32, 512, 512, 5
S = W // 2
P = 128
KC = E // P            # feature chunks of 128
B_LOC = B // N_CORES   # batches per core
EPS = 1e-3
F32 = mybir.dt.float32
AF = mybir.ActivationFunctionType
ALU = mybir.AluOpType
AX = mybir.AxisListType

# gpack column base offsets (each vector packed as [P, KC])
_G1, _B1, _G2, _B2, _G3, _B3, _G4, _B4, _G5, _B5 = (i * KC for i in range(10))

_MAX_CTRL_WAITS = 1


def _split_waits(nc, max_waits=_MAX_CTRL_WAITS):
    """walrus in this container encodes at most one sync-wait slot per
    instruction. Hoist extra waits onto same-engine NOPs inserted right
    before the owning instruction (same engine => executes first)."""
    for fn in nc.m.functions:
        for bb in fn.blocks:
            rebuilt = []
            changed = False
            for ins in bb.instructions:
                si = ins.sync_info
                if si is not None and len(si.on_wait) > max_waits:
                    waits = list(si.on_wait)
                    rest = waits[max_waits:]
                    for j in range(0, len(rest), max_waits):
                        nop = mybir.InstNoOp(
                            name=f"{ins.name}_wsplit{j}",
                            engine=ins.engine,
                            bass_nofuse=True,
                            sync_info=mybir.SyncInfo(
                                on_wait=rest[j : j + max_waits], on_update=[]
                            ),
                        )
                        rebuilt.append(nop)
                    ins.sync_info = mybir.SyncInfo(
                        on_wait=waits[:max_waits], on_update=list(si.on_update)
                    )
                    changed = True
                rebuilt.append(ins)
            if changed:
                bb.instructions = rebuilt


_CACHE = {}


def _build():
    if "nc" in _CACHE:
        return _CACHE["nc"]
    nc = bass.Bass("TRN2", target_bir_lowering=False, debug=False, num_devices=N_CORES)

    m1t_d = nc.dram_tensor("m1t", [B_LOC, E, L], F32, kind="ExternalInput")
    f_d = nc.dram_tensor("f", [W * E, E], F32, kind="ExternalInput")
    wq_d = nc.dram_tensor("wq", [E, E], F32, kind="ExternalInput")
    wk_d = nc.dram_tensor("wk", [E, E], F32, kind="ExternalInput")
    qbt_d = nc.dram_tensor("qbt", [E, L], F32, kind="ExternalInput")
    kbt_d = nc.dram_tensor("kbt", [E, L], F32, kind="ExternalInput")
    wbt_d = nc.dram_tensor("wbt", [L, L], F32, kind="ExternalInput")
    gp_d = nc.dram_tensor("gpack", [P, 10 * KC], F32, kind="ExternalInput")
    out_d = nc.dram_tensor("outt", [B_LOC, E, L], F32, kind="ExternalOutput")

    groups = [list(range(N_CORES))]

    from contextlib import ExitStack

    with tile.TileContext(nc) as tc:
        with (
            tc.tile_pool(name="const", bufs=1) as const,
            tc.tile_pool(name="bias", bufs=4) as biasp,
            tc.tile_pool(name="aff", bufs=44) as affp,
            tc.tile_pool(name="stats", bufs=24) as statp,
            tc.tile_pool(name="packs", bufs=8) as packp,
            tc.tile_pool(name="scr", bufs=16) as scr,
            tc.tile_pool(name="elu", bufs=3) as elup,
            tc.tile_pool(name="psum", bufs=4, space="PSUM") as psum,
            tc.tile_pool(name="psumT", bufs=4, space="PSUM") as psumT,
            tc.tile_pool(name="dram", bufs=8, space="DRAM") as dram,
        ):
            es_l = ExitStack()
            wtp = es_l.enter_context(tc.tile_pool(name="wT", bufs=B_LOC * KC))
            lp = es_l.enter_context(tc.tile_pool(name="l", bufs=B_LOC * KC))
            gp = const.tile([P, 10 * KC], F32, tag="gp")
            nc.sync.dma_start(out=gp[:], in_=gp_d[:])
            ident = const.tile([P, P], F32, tag="ident")
            make_identity(nc, ident[:])
            epst = const.tile([P, 1], F32, tag="eps")
            nc.vector.memset(epst[:], EPS)

            qbt_sb, kbt_sb, wbt_sb = {}, {}, {}
            for c in range(KC):
                t = biasp.tile([P, L], F32, tag="qbt")
                nc.sync.dma_start(out=t[:], in_=qbt_d[c * P : (c + 1) * P, :])
                qbt_sb[c] = t
                t = biasp.tile([P, L], F32, tag="kbt")
                nc.sync.dma_start(out=t[:], in_=kbt_d[c * P : (c + 1) * P, :])
                kbt_sb[c] = t
                t = biasp.tile([P, L], F32, tag="wbt")
                nc.sync.dma_start(out=t[:], in_=wbt_d[c * P : (c + 1) * P, :])
                wbt_sb[c] = t

            def stats_to_pack(stats_tiles, pack):
                """stats_tiles: per-chunk [P, B_LOC, 6] bn_stats rows.
                pack[:, c, 0] = local_mean/8, pack[:, c, 1] = local_E[x^2]/8."""
                for c, st in enumerate(stats_tiles):
                    mv = scr.tile([P, 2], F32, tag="scr")
                    nc.vector.bn_aggr(out=mv[:], in_=st[:])
                    sq = scr.tile([P, 1], F32, tag="scr")
                    nc.vector.tensor_mul(sq[:], mv[:, 0:1], mv[:, 0:1])
                    nc.vector.tensor_copy(pack[:, c, 0:1], mv[:, 0:1])
                    nc.vector.tensor_add(pack[:, c, 1:2], mv[:, 1:2], sq[:])
                nc.vector.tensor_scalar_mul(pack[:], pack[:], 1.0 / N_CORES)

            def allreduce(pack, width):
                cc_in = dram.tile([P, width], F32, tag="cc")
                cc_out = dram.tile([P, width], F32, tag="cc")
                nc.gpsimd.dma_start(out=cc_in[:], in_=pack[:])
                nc.gpsimd.collective_compute(
                    "AllReduce",
                    ALU.add,
                    replica_groups=groups,
                    ins=[cc_in.opt()],
                    outs=[cc_out.opt()],
                )
                g = packp.tile([P, width], F32, tag="g")
                nc.gpsimd.dma_start(out=g[:], in_=cc_out[:])
                return g

            def affines(g, gcol, bcol):
                """From allreduced [P, KC, 2] (mean, E[x^2]) compute per-chunk
                scale = gamma*rsqrt(var+eps), bias = beta - mean*scale."""
                sc_l, bi_l = [], []
                gv = g.rearrange("p (c two) -> p c two", two=2)
                for c in range(KC):
                    mean = gv[:, c, 0:1]
                    ex2 = gv[:, c, 1:2]
                    sq = scr.tile([P, 1], F32, tag="scr")
                    nc.vector.tensor_mul(sq[:], mean, mean)
                    var = scr.tile([P, 1], F32, tag="scr")
                    nc.vector.tensor_sub(var[:], ex2, sq[:])
                    sd = scr.tile([P, 1], F32, tag="scr")
                    nc.scalar.activation(out=sd[:], in_=var[:], func=AF.Sqrt, bias=epst[:])
                    rinv = scr.tile([P, 1], F32, tag="scr")
                    nc.vector.reciprocal(rinv[:], sd[:])
                    sc = affp.tile([P, 1], F32, tag="aff")
                    nc.vector.tensor_mul(sc[:], rinv[:], gp[:, gcol + c : gcol + c + 1])
                    tb = scr.tile([P, 1], F32, tag="scr")
                    nc.vector.tensor_mul(tb[:], mean, sc[:])
                    bi = affp.tile([P, 1], F32, tag="aff")
                    nc.vector.tensor_sub(bi[:], gp[:, bcol + c : bcol + c + 1], tb[:])
                    sc_l.append(sc)
                    bi_l.append(bi)
                return sc_l, bi_l

            def elu_apply(zt, sc, bi):
                """zt <- elu(zt*sc + bi) = relu(y) + (min(exp(y),1) - 1)."""
                e = elup.tile([P, L], F32, tag="elu_e")
                r = elup.tile([P, L], F32, tag="elu_r")
                nc.scalar.activation(out=e[:], in_=zt[:], func=AF.Exp, bias=bi[:], scale=sc[:])
                nc.scalar.activation(out=r[:], in_=zt[:], func=AF.Relu, bias=bi[:], scale=sc[:])
                nc.vector.tensor_scalar(
                    out=e[:], in0=e[:], scalar1=1.0, scalar2=1.0,
                    op0=ALU.min, op1=ALU.subtract,
                )
                nc.vector.tensor_tensor(out=zt[:], in0=r[:], in1=e[:], op=ALU.add)

            # ---------------- Stage 1: z1 = unfold(m1) @ f + kb ----------------
            l_sb = {}
            stats1 = [statp.tile([P, B_LOC, 6], F32, tag="st", name="st1") for _ in range(KC)]
            es_s1 = ExitStack()
            if True:
                fp = es_s1.enter_context(tc.tile_pool(name="f", bufs=W * KC))
                mp = es_s1.enter_context(tc.tile_pool(name="m1", bufs=B_LOC * KC))
                f_sb = {}
                for w in range(W):
                    for kc in range(KC):
                        t = fp.tile([P, E], F32, tag="f")
                        r0 = (w * KC + kc) * P
                        nc.sync.dma_start(out=t[:], in_=f_d[r0 : r0 + P, :])
                        f_sb[w, kc] = t
                m1_sb = {}
                for b in range(B_LOC):
                    for kc in range(KC):
                        t = mp.tile([P, L + 2 * S], F32, tag="m1")
                        nc.vector.memset(t[:, 0:S], 0.0)
                        nc.vector.memset(t[:, S + L : 2 * S + L], 0.0)
                        nc.sync.dma_start(
                            out=t[:, S : S + L], in_=m1t_d[b, kc * P : (kc + 1) * P, :]
                        )
                        m1_sb[b, kc] = t

                for b in range(B_LOC):
                    for mc in range(KC):
                        ps = psum.tile([P, L], F32, tag="ps")
                        n = 0
                        for w in range(W):
                            for kc in range(KC):
                                nc.tensor.matmul(
                                    ps[:],
                                    f_sb[w, kc][:, mc * P : (mc + 1) * P],
                                    m1_sb[b, kc][:, w : w + L],
                                    start=(n == 0),
                                    stop=(n == W * KC - 1),
                                )
                                n += 1
                        zt = lp.tile([P, L], F32, tag="l")
                        nc.vector.tensor_tensor(out=zt[:], in0=ps[:], in1=kbt_sb[mc][:], op=ALU.add)
                        nc.vector.bn_stats(out=stats1[mc][:, b, :], in_=zt[:])
                        l_sb[b, mc] = zt

            es_s1.close()

            pack1 = packp.tile([P, KC, 2], F32, tag="g")
            stats_to_pack(stats1, pack1)
            g1 = allreduce(pack1, KC * 2)
            sc1, bi1 = affines(g1, _G1, _B1)
            for b in range(B_LOC):
                for mc in range(KC):
                    elu_apply(l_sb[b, mc], sc1[mc], bi1[mc])

            # ------------- Stage 2/3: q2 = l@wq + qb, k2 = l@wk + kb -------------
            stats2 = [statp.tile([P, B_LOC, 6], F32, tag="st", name="st2") for _ in range(KC)]
            stats3 = [statp.tile([P, B_LOC, 6], F32, tag="st", name="st3") for _ in range(KC)]
            es_wqk = ExitStack()
            es_z = ExitStack()
            if True:
                z2p = es_z.enter_context(tc.tile_pool(name="z2", bufs=B_LOC * KC))
                z3p = es_z.enter_context(tc.tile_pool(name="z3", bufs=B_LOC * KC))
                wqkp = es_wqk.enter_context(tc.tile_pool(name="wqk", bufs=2 * KC))
                wq_sb, wk_sb = {}, {}
                for kc in range(KC):
                    t = wqkp.tile([P, E], F32, tag="wqk")
                    nc.sync.dma_start(out=t[:], in_=wq_d[kc * P : (kc + 1) * P, :])
                    wq_sb[kc] = t
                    t = wqkp.tile([P, E], F32, tag="wqk")
                    nc.sync.dma_start(out=t[:], in_=wk_d[kc * P : (kc + 1) * P, :])
                    wk_sb[kc] = t

                q2_sb, k2_sb = {}, {}
                for b in range(B_LOC):
                    for mc in range(KC):
                        ps = psum.tile([P, L], F32, tag="ps")
                        for kc in range(KC):
                            nc.tensor.matmul(
                                ps[:],
                                wq_sb[kc][:, mc * P : (mc + 1) * P],
                                l_sb[b, kc][:],
                                start=(kc == 0),
                                stop=(kc == KC - 1),
                            )
                        zt = z2p.tile([P, L], F32, tag="z2")
                        nc.vector.tensor_tensor(out=zt[:], in0=ps[:], in1=qbt_sb[mc][:], op=ALU.add)
                        nc.vector.bn_stats(out=stats2[mc][:, b, :], in_=zt[:])
                        q2_sb[b, mc] = zt

                        ps = psum.tile([P, L], F32, tag="ps")
                        for kc in range(KC):
                            nc.tensor.matmul(
                                ps[:],
                                wk_sb[kc][:, mc * P : (mc + 1) * P],
                                l_sb[b, kc][:],
                                start=(kc == 0),
                                stop=(kc == KC - 1),
                            )
                        zt = z3p.tile([P, L], F32, tag="z3")
                        nc.vector.tensor_tensor(out=zt[:], in0=ps[:], in1=kbt_sb[mc][:], op=ALU.add)
                        nc.vector.bn_stats(out=stats3[mc][:, b, :], in_=zt[:])
                        k2_sb[b, mc] = zt

                pack23 = packp.tile([P, 2 * KC, 2], F32, tag="g")
                for c, st in enumerate(stats2 + stats3):
                    mv = scr.tile([P, 2], F32, tag="scr")
                    nc.vector.bn_aggr(out=mv[:], in_=st[:])
                    sq = scr.tile([P, 1], F32, tag="scr")
                    nc.vector.tensor_mul(sq[:], mv[:, 0:1], mv[:, 0:1])
                    nc.vector.tensor_copy(pack23[:, c, 0:1], mv[:, 0:1])
                    nc.vector.tensor_add(pack23[:, c, 1:2], mv[:, 1:2], sq[:])
                nc.vector.tensor_scalar_mul(pack23[:], pack23[:], 1.0 / N_CORES)
                g23 = allreduce(pack23, 4 * KC)
                sc2, bi2 = affines(g23[:, 0 : 2 * KC], _G2, _B2)
                sc3, bi3 = affines(g23[:, 2 * KC : 4 * KC], _G3, _B3)

                for b in range(B_LOC):
                    for mc in range(KC):
                        elu_apply(q2_sb[b, mc], sc2[mc], bi2[mc])
                        elu_apply(k2_sb[b, mc], sc3[mc], bi3[mc])

                # ------------- Stage 4a: wT = (q2 @ k2^T)^T + wb^T -------------
                es_wqk.close()
                stats4 = [statp.tile([P, B_LOC, 6], F32, tag="st", name="st4") for _ in range(KC)]
                wt_sb = {}
                for b in range(B_LOC):
                    for kc in range(KC):
                        ps = psum.tile([P, L], F32, tag="ps")
                        for ec in range(KC):
                            nc.tensor.matmul(
                                ps[:],
                                k2_sb[b, ec][:, kc * P : (kc + 1) * P],
                                q2_sb[b, ec][:],
                                start=(ec == 0),
                                stop=(ec == KC - 1),
                            )
                        wt = wtp.tile([P, L], F32, tag="wT")
                        nc.vector.tensor_tensor(out=wt[:], in0=ps[:], in1=wbt_sb[kc][:], op=ALU.add)
                        nc.vector.bn_stats(out=stats4[kc][:, b, :], in_=wt[:])
                        wt_sb[b, kc] = wt

            es_z.close()

            pack4 = packp.tile([P, KC, 2], F32, tag="g")
            stats_to_pack(stats4, pack4)
            g4 = allreduce(pack4, KC * 2)
            sc4, bi4 = affines(g4, _G4, _B4)

            # ---------------- Stage 4b: BN4 + softmax over q ----------------
            for b in range(B_LOC):
                for kc in range(KC):
                    t = wt_sb[b, kc]
                    nc.vector.tensor_scalar(
                        out=t[:], in0=t[:], scalar1=sc4[kc][:], scalar2=bi4[kc][:],
                        op0=ALU.mult, op1=ALU.add,
                    )
                    mx = scr.tile([P, 1], F32, tag="scr")
                    nc.vector.tensor_reduce(out=mx[:], in_=t[:], axis=AX.X, op=ALU.max)
                    nm = scr.tile([P, 1], F32, tag="scr")
                    nc.vector.tensor_scalar_mul(nm[:], mx[:], -1.0)
                    ssum = scr.tile([P, 1], F32, tag="scr")
                    nc.scalar.activation(
                        out=t[:], in_=t[:], func=AF.Exp, bias=nm[:], accum_out=ssum[:]
                    )
                    rs = scr.tile([P, 1], F32, tag="scr")
                    nc.vector.reciprocal(rs[:], ssum[:])
                    nc.vector.tensor_scalar_mul(t[:], t[:], rs[:])

            # ---------------- Stage 5: out = w @ l, BN5 + ELU ----------------
            stats5 = [statp.tile([P, B_LOC, 6], F32, tag="st", name="st5") for _ in range(KC)]
            es_s5 = ExitStack()
            if True:
                lsp = es_s5.enter_context(tc.tile_pool(name="lstd", bufs=B_LOC * KC))
                outp = es_s5.enter_context(tc.tile_pool(name="out", bufs=B_LOC * KC))
                lstd_sb = {}
                for b in range(B_LOC):
                    for kc in range(KC):
                        lst = lsp.tile([P, E], F32, tag="lstd")
                        for mc in range(KC):
                            pst = psumT.tile([P, P], F32, tag="psT")
                            nc.tensor.transpose(
                                pst[:], l_sb[b, mc][:, kc * P : (kc + 1) * P], ident[:]
                            )
                            nc.vector.tensor_copy(lst[:, mc * P : (mc + 1) * P], pst[:])
                        lstd_sb[b, kc] = lst

                out_sb = {}
                for b in range(B_LOC):
                    for mc in range(KC):
                        ps = psum.tile([P, L], F32, tag="ps")
                        for kc in range(KC):
                            nc.tensor.matmul(
                                ps[:],
                                lstd_sb[b, kc][:, mc * P : (mc + 1) * P],
                                wt_sb[b, kc][:],
                                start=(kc == 0),
                                stop=(kc == KC - 1),
                            )
                        ot = outp.tile([P, L], F32, tag="out")
                        nc.vector.tensor_copy(ot[:], ps[:])
                        nc.vector.bn_stats(out=stats5[mc][:, b, :], in_=ot[:])
                        out_sb[b, mc] = ot

                pack5 = packp.tile([P, KC, 2], F32, tag="g")
                stats_to_pack(stats5, pack5)
                g5 = allreduce(pack5, KC * 2)
                sc5, bi5 = affines(g5, _G5, _B5)
                for b in range(B_LOC):
                    for mc in range(KC):
                        elu_apply(out_sb[b, mc], sc5[mc], bi5[mc])
                        nc.sync.dma_start(
                            out=out_d[b, mc * P : (mc + 1) * P, :], in_=out_sb[b, mc][:]
                        )

                es_s5.close()
                es_l.close()

    _split_waits(nc)
    _CACHE["nc"] = nc
    return nc


def _pack_affine(vecs):
    cols = []
    for v in vecs:
        cols.append(np.ascontiguousarray(np.asarray(v, np.float32).reshape(KC, P).T))
    return np.ascontiguousarray(np.concatenate(cols, axis=1))


def kernel(m1, f, wq, wk, qb, kb, wb, g1, b1, g2, b2, g3, b3, g4, b4, g5, b5):
    m1 = np.asarray(m1, np.float32)
    nc = _build()
    m1t = np.ascontiguousarray(m1.transpose(0, 2, 1))
    f_h = np.ascontiguousarray(np.asarray(f, np.float32))
    wq_h = np.ascontiguousarray(np.asarray(wq, np.float32))
    wk_h = np.ascontiguousarray(np.asarray(wk, np.float32))
    qbt = np.ascontiguousarray(np.asarray(qb, np.float32).T)
    kbt = np.ascontiguousarray(np.asarray(kb, np.float32).T)
    wbt = np.ascontiguousarray(np.asarray(wb, np.float32).T)
    gpack = _pack_affine([g1, b1, g2, b2, g3, b3, g4, b4, g5, b5])

    shared = {
        "f": f_h, "wq": wq_h, "wk": wk_h,
        "qbt": qbt, "kbt": kbt, "wbt": wbt, "gpack": gpack,
    }
    in_maps = [
        {"m1t": np.ascontiguousarray(m1t[i * B_LOC : (i + 1) * B_LOC]), **shared}
        for i in range(N_CORES)
    ]
    trace = os.environ.get("KERNEL_TRACE") == "1"
    res = run_bass_kernel_spmd(nc, in_maps, list(range(N_CORES)), trace=trace)
    _CACHE["last_results"] = res

    out = np.empty((B, L, E), np.float32)
    for i in range(N_CORES):
        out[i * B_LOC : (i + 1) * B_LOC] = res.results[i]["outt"].transpose(0, 2, 1)
    return out



# revision 28
# speedup vs baseline: 1.9405x; 1.9405x over previous
"""Bass/Tile TRN2 kernel for nn_LAN_4320737100678 (dense transformer block).

Data-parallel over the batch axis across 8 NeuronCores (4 batches/core).
All activations are kept feature-major ([E, L] per batch) so that every
BatchNorm reduction and the softmax run along the free axis, and the BN
affine+ELU applications are single activation-engine passes with
per-partition scale/bias. The five BatchNorm moment sets are globalized
with four tiny in-kernel AllReduces (BN2+BN3 share one round).

The sliding-window "unfold" (W=5) is never materialized: stage 1 is
computed as 5 shifted matmuls accumulating into PSUM from a zero-padded
copy of m1^T.
"""

import os
import sys

sys.path.insert(0, "/opt/trn_rl_repo")

import numpy as np

import concourse.bass as bass
import concourse.tile as tile
from concourse import mybir
from concourse.bass_utils import run_bass_kernel_spmd
from concourse.masks import make_identity
from concourse.vector_clock import ScopedClock

N_CORES = 8
B, L, E, W = 32, 512, 512, 5
S = W // 2
P = 128
KC = E // P            # feature chunks of 128
B_LOC = B // N_CORES   # batches per core
EPS = 1e-3
F32 = mybir.dt.float32
F32R = mybir.dt.float32r
AF = mybir.ActivationFunctionType
ALU = mybir.AluOpType
AX = mybir.AxisListType

# gpack column base offsets (each vector packed as [P, KC])
_G1, _B1, _G2, _B2, _G3, _B3, _G4, _B4, _G5, _B5 = (i * KC for i in range(10))

_MAX_CTRL_WAITS = 1


def _split_waits(nc, max_waits=_MAX_CTRL_WAITS):
    """walrus in this container encodes at most one sync-wait slot per
    instruction. Hoist extra waits onto same-engine NOPs inserted right
    before the owning instruction (same engine => executes first)."""
    for fn in nc.m.functions:
        for bb in fn.blocks:
            rebuilt = []
            changed = False
            for ins in bb.instructions:
                si = ins.sync_info
                if si is not None and len(si.on_wait) > max_waits:
                    waits = list(si.on_wait)
                    rest = waits[max_waits:]
                    for j in range(0, len(rest), max_waits):
                        nop = mybir.InstNoOp(
                            name=f"{ins.name}_wsplit{j}",
                            engine=ins.engine,
                            bass_nofuse=True,
                            sync_info=mybir.SyncInfo(
                                on_wait=rest[j : j + max_waits], on_update=[]
                            ),
                        )
                        rebuilt.append(nop)
                    ins.sync_info = mybir.SyncInfo(
                        on_wait=waits[:max_waits], on_update=list(si.on_update)
                    )
                    changed = True
                rebuilt.append(ins)
            if changed:
                bb.instructions = rebuilt


_CACHE = {}


def _build():
    if "nc" in _CACHE:
        return _CACHE["nc"]
    nc = bass.Bass("TRN2", target_bir_lowering=False, debug=False, num_devices=N_CORES)

    m1t_d = nc.dram_tensor("m1t", [B_LOC, E, L + 2 * S], F32, kind="ExternalInput")
    f_d = nc.dram_tensor("f", [W * E, E], F32, kind="ExternalInput")
    wq_d = nc.dram_tensor("wq", [E, E], F32, kind="ExternalInput")
    wk_d = nc.dram_tensor("wk", [E, E], F32, kind="ExternalInput")
    qbt_d = nc.dram_tensor("qbt", [E, L], F32, kind="ExternalInput")
    kbt_d = nc.dram_tensor("kbt", [E, L], F32, kind="ExternalInput")
    wbt_d = nc.dram_tensor("wbt", [L, L], F32, kind="ExternalInput")
    gp_d = nc.dram_tensor("gpack", [P, 10 * KC], F32, kind="ExternalInput")
    out_d = nc.dram_tensor("outt", [B_LOC, E, L], F32, kind="ExternalOutput")

    groups = [list(range(N_CORES))]

    from contextlib import ExitStack

    with tile.TileContext(nc) as tc:
        with (
            tc.tile_pool(name="const", bufs=1) as const,
            tc.tile_pool(name="bias", bufs=4) as biasp,
            tc.tile_pool(name="aff", bufs=44) as affp,
            tc.tile_pool(name="stats", bufs=24) as statp,
            tc.tile_pool(name="packs", bufs=8) as packp,
            tc.tile_pool(name="scr", bufs=16) as scr,
            tc.tile_pool(name="elu", bufs=3) as elup,
            tc.tile_pool(name="psum", bufs=4, space="PSUM") as psum,
            tc.tile_pool(name="psumT", bufs=4, space="PSUM") as psumT,
            tc.tile_pool(name="dram", bufs=8, space="DRAM") as dram,
        ):
            def mmr(ps, lhsT, rhs, start, stop):
                """fp32 matmul issued as float32r: single-pass PE (1 cycle/row
                at free>=256) instead of fp32's two half-speed passes. Producers
                of both operands must write f32r-tagged outputs (BIR verifier)."""
                nc.tensor.matmul(
                    ps,
                    lhsT.bitcast(F32R),
                    rhs.bitcast(F32R),
                    start=start,
                    stop=stop,
                )

            es_l = ExitStack()
            wtp = es_l.enter_context(tc.tile_pool(name="wT", bufs=B_LOC * KC))
            lp = es_l.enter_context(tc.tile_pool(name="l", bufs=B_LOC * KC))
            gp = const.tile([P, 10 * KC], F32, tag="gp")
            nc.sync.dma_start(out=gp[:], in_=gp_d[:])
            ident = const.tile([P, P], F32, tag="ident")
            make_identity(nc, ident[:])
            epst = const.tile([P, 1], F32, tag="eps")
            nc.vector.memset(epst[:], EPS)

            qbt_sb, kbt_sb, wbt_sb = {}, {}, {}
            for c in range(KC):
                t = biasp.tile([P, L], F32, tag="qbt")
                nc.sync.dma_start(out=t[:], in_=qbt_d[c * P : (c + 1) * P, :])
                qbt_sb[c] = t
                t = biasp.tile([P, L], F32, tag="kbt")
                nc.sync.dma_start(out=t[:], in_=kbt_d[c * P : (c + 1) * P, :])
                kbt_sb[c] = t
                t = biasp.tile([P, L], F32, tag="wbt")
                nc.sync.dma_start(out=t[:], in_=wbt_d[c * P : (c + 1) * P, :])
                wbt_sb[c] = t

            def stats_to_pack(stats_tiles, pack):
                """stats_tiles: per-chunk [P, B_LOC, 6] bn_stats rows.
                pack[:, c, 0] = local_mean/8, pack[:, c, 1] = local_E[x^2]/8."""
                for c, st in enumerate(stats_tiles):
                    mv = scr.tile([P, 2], F32, tag="scr")
                    nc.vector.bn_aggr(out=mv[:], in_=st[:])
                    sq = scr.tile([P, 1], F32, tag="scr")
                    nc.vector.tensor_mul(sq[:], mv[:, 0:1], mv[:, 0:1])
                    nc.vector.tensor_copy(pack[:, c, 0:1], mv[:, 0:1])
                    nc.vector.tensor_add(pack[:, c, 1:2], mv[:, 1:2], sq[:])
                nc.vector.tensor_scalar_mul(pack[:], pack[:], 1.0 / N_CORES)

            def allreduce(pack, width):
                cc_in = dram.tile([P, width], F32, tag="cc")
                cc_out = dram.tile([P, width], F32, tag="cc")
                nc.gpsimd.dma_start(out=cc_in[:], in_=pack[:])
                nc.gpsimd.collective_compute(
                    "AllReduce",
                    ALU.add,
                    replica_groups=groups,
                    ins=[cc_in.opt()],
                    outs=[cc_out.opt()],
                )
                g = packp.tile([P, width], F32, tag="g")
                nc.gpsimd.dma_start(out=g[:], in_=cc_out[:])
                return g

            def affines(g, gcol, bcol):
                """From allreduced [P, KC, 2] (mean, E[x^2]) compute per-chunk
                scale = gamma*rsqrt(var+eps), bias = beta - mean*scale."""
                sc_l, bi_l = [], []
                gv = g.rearrange("p (c two) -> p c two", two=2)
                for c in range(KC):
                    mean = gv[:, c, 0:1]
                    ex2 = gv[:, c, 1:2]
                    sq = scr.tile([P, 1], F32, tag="scr")
                    nc.vector.tensor_mul(sq[:], mean, mean)
                    var = scr.tile([P, 1], F32, tag="scr")
                    nc.vector.tensor_sub(var[:], ex2, sq[:])
                    sd = scr.tile([P, 1], F32, tag="scr")
                    nc.scalar.activation(out=sd[:], in_=var[:], func=AF.Sqrt, bias=epst[:])
                    rinv = scr.tile([P, 1], F32, tag="scr")
                    nc.vector.reciprocal(rinv[:], sd[:])
                    sc = affp.tile([P, 1], F32, tag="aff")
                    nc.vector.tensor_mul(sc[:], rinv[:], gp[:, gcol + c : gcol + c + 1])
                    tb = scr.tile([P, 1], F32, tag="scr")
                    nc.vector.tensor_mul(tb[:], mean, sc[:])
                    bi = affp.tile([P, 1], F32, tag="aff")
                    nc.vector.tensor_sub(bi[:], gp[:, bcol + c : bcol + c + 1], tb[:])
                    sc_l.append(sc)
                    bi_l.append(bi)
                return sc_l, bi_l

            def elu_apply(zt, sc, bi, round_out=True):
                """zt <- elu(zt*sc + bi) = relu(y) + (min(exp(y),1) - 1).
                round_out tags the final write float32r so downstream f32r
                matmuls pass BIR verification."""
                e = elup.tile([P, L], F32, tag="elu_e")
                r = elup.tile([P, L], F32, tag="elu_r")
                nc.scalar.activation(out=e[:], in_=zt[:], func=AF.Exp, bias=bi[:], scale=sc[:])
                nc.scalar.activation(out=r[:], in_=zt[:], func=AF.Relu, bias=bi[:], scale=sc[:])
                nc.vector.tensor_scalar(
                    out=e[:], in0=e[:], scalar1=1.0, scalar2=1.0,
                    op0=ALU.min, op1=ALU.subtract,
                )
                out_ap = zt[:].bitcast(F32R) if round_out else zt[:]
                nc.vector.tensor_tensor(out=out_ap, in0=r[:], in1=e[:], op=ALU.add)

            # ---------------- Stage 1: z1 = unfold(m1) @ f + kb ----------------
            l_sb = {}
            stats1 = [statp.tile([P, B_LOC, 6], F32, tag="st", name="st1") for _ in range(KC)]
            es_s1 = ExitStack()
            if True:
                fp = es_s1.enter_context(tc.tile_pool(name="f", bufs=W * KC))
                mp = es_s1.enter_context(tc.tile_pool(name="m1", bufs=B_LOC * KC))
                f_sb = {}
                for w in range(W):
                    for kc in range(KC):
                        t = fp.tile([P, E], F32, tag="f")
                        r0 = (w * KC + kc) * P
                        nc.sync.dma_start(
                            out=t[:].bitcast(F32R), in_=f_d[r0 : r0 + P, :].bitcast(F32R)
                        )
                        f_sb[w, kc] = t
                m1_sb = {}
                for b in range(B_LOC):
                    for kc in range(KC):
                        t = mp.tile([P, L + 2 * S], F32, tag="m1")
                        nc.sync.dma_start(
                            out=t[:].bitcast(F32R),
                            in_=m1t_d[b, kc * P : (kc + 1) * P, :].bitcast(F32R),
                        )
                        m1_sb[b, kc] = t

                for b in range(B_LOC):
                    for mc in range(KC):
                        ps = psum.tile([P, L], F32, tag="ps")
                        n = 0
                        for w in range(W):
                            for kc in range(KC):
                                mmr(
                                    ps[:],
                                    f_sb[w, kc][:, mc * P : (mc + 1) * P],
                                    m1_sb[b, kc][:, w : w + L],
                                    start=(n == 0),
                                    stop=(n == W * KC - 1),
                                )
                                n += 1
                        zt = lp.tile([P, L], F32, tag="l")
                        nc.vector.tensor_tensor(
                            out=zt[:].bitcast(F32R), in0=ps[:], in1=kbt_sb[mc][:], op=ALU.add
                        )
                        nc.vector.bn_stats(out=stats1[mc][:, b, :], in_=zt[:])
                        l_sb[b, mc] = zt

            es_s1.close()

            pack1 = packp.tile([P, KC, 2], F32, tag="g")
            stats_to_pack(stats1, pack1)
            g1 = allreduce(pack1, KC * 2)
            sc1, bi1 = affines(g1, _G1, _B1)
            for b in range(B_LOC):
                for mc in range(KC):
                    elu_apply(l_sb[b, mc], sc1[mc], bi1[mc])

            # ------------- Stage 2/3: q2 = l@wq + qb, k2 = l@wk + kb -------------
            stats2 = [statp.tile([P, B_LOC, 6], F32, tag="st", name="st2") for _ in range(KC)]
            stats3 = [statp.tile([P, B_LOC, 6], F32, tag="st", name="st3") for _ in range(KC)]
            es_wqk = ExitStack()
            es_z = ExitStack()
            if True:
                z2p = es_z.enter_context(tc.tile_pool(name="z2", bufs=B_LOC * KC))
                z3p = es_z.enter_context(tc.tile_pool(name="z3", bufs=B_LOC * KC))
                wqkp = es_wqk.enter_context(tc.tile_pool(name="wqk", bufs=2 * KC))
                wq_sb, wk_sb = {}, {}
                for kc in range(KC):
                    t = wqkp.tile([P, E], F32, tag="wqk")
                    nc.sync.dma_start(
                        out=t[:].bitcast(F32R),
                        in_=wq_d[kc * P : (kc + 1) * P, :].bitcast(F32R),
                    )
                    wq_sb[kc] = t
                    t = wqkp.tile([P, E], F32, tag="wqk")
                    nc.sync.dma_start(
                        out=t[:].bitcast(F32R),
                        in_=wk_d[kc * P : (kc + 1) * P, :].bitcast(F32R),
                    )
                    wk_sb[kc] = t

                q2_sb, k2_sb = {}, {}
                for b in range(B_LOC):
                    for mc in range(KC):
                        ps = psum.tile([P, L], F32, tag="ps")
                        for kc in range(KC):
                            mmr(
                                ps[:],
                                wq_sb[kc][:, mc * P : (mc + 1) * P],
                                l_sb[b, kc][:],
                                start=(kc == 0),
                                stop=(kc == KC - 1),
                            )
                        zt = z2p.tile([P, L], F32, tag="z2")
                        nc.vector.tensor_tensor(
                            out=zt[:].bitcast(F32R), in0=ps[:], in1=qbt_sb[mc][:], op=ALU.add
                        )
                        nc.vector.bn_stats(out=stats2[mc][:, b, :], in_=zt[:])
                        q2_sb[b, mc] = zt

                        ps = psum.tile([P, L], F32, tag="ps")
                        for kc in range(KC):
                            mmr(
                                ps[:],
                                wk_sb[kc][:, mc * P : (mc + 1) * P],
                                l_sb[b, kc][:],
                                start=(kc == 0),
                                stop=(kc == KC - 1),
                            )
                        zt = z3p.tile([P, L], F32, tag="z3")
                        nc.vector.tensor_tensor(
                            out=zt[:].bitcast(F32R), in0=ps[:], in1=kbt_sb[mc][:], op=ALU.add
                        )
                        nc.vector.bn_stats(out=stats3[mc][:, b, :], in_=zt[:])
                        k2_sb[b, mc] = zt

                pack23 = packp.tile([P, 2 * KC, 2], F32, tag="g")
                for c, st in enumerate(stats2 + stats3):
                    mv = scr.tile([P, 2], F32, tag="scr")
                    nc.vector.bn_aggr(out=mv[:], in_=st[:])
                    sq = scr.tile([P, 1], F32, tag="scr")
                    nc.vector.tensor_mul(sq[:], mv[:, 0:1], mv[:, 0:1])
                    nc.vector.tensor_copy(pack23[:, c, 0:1], mv[:, 0:1])
                    nc.vector.tensor_add(pack23[:, c, 1:2], mv[:, 1:2], sq[:])
                nc.vector.tensor_scalar_mul(pack23[:], pack23[:], 1.0 / N_CORES)
                g23 = allreduce(pack23, 4 * KC)
                sc2, bi2 = affines(g23[:, 0 : 2 * KC], _G2, _B2)
                sc3, bi3 = affines(g23[:, 2 * KC : 4 * KC], _G3, _B3)

                for b in range(B_LOC):
                    for mc in range(KC):
                        elu_apply(q2_sb[b, mc], sc2[mc], bi2[mc])
                        elu_apply(k2_sb[b, mc], sc3[mc], bi3[mc])

                # ------------- Stage 4a: wT = (q2 @ k2^T)^T + wb^T -------------
                es_wqk.close()
                es_w4 = ExitStack()
                wrawp = es_w4.enter_context(tc.tile_pool(name="wraw", bufs=B_LOC * KC))
                stats4 = [statp.tile([P, B_LOC, 6], F32, tag="st", name="st4") for _ in range(KC)]
                wraw_sb = {}
                for b in range(B_LOC):
                    for kc in range(KC):
                        ps = psum.tile([P, L], F32, tag="ps")
                        for ec in range(KC):
                            mmr(
                                ps[:],
                                k2_sb[b, ec][:, kc * P : (kc + 1) * P],
                                q2_sb[b, ec][:],
                                start=(ec == 0),
                                stop=(ec == KC - 1),
                            )
                        wt = wrawp.tile([P, L], F32, tag="wraw")
                        nc.vector.tensor_tensor(out=wt[:], in0=ps[:], in1=wbt_sb[kc][:], op=ALU.add)
                        nc.vector.bn_stats(out=stats4[kc][:, b, :], in_=wt[:])
                        wraw_sb[b, kc] = wt

            pack4 = packp.tile([P, KC, 2], F32, tag="g")
            stats_to_pack(stats4, pack4)
            g4 = allreduce(pack4, KC * 2)
            sc4, bi4 = affines(g4, _G4, _B4)

            # ---------------- Stage 4b: BN4 + softmax over q ----------------
            # softmax(bn4(w)) over the free axis, with the BN affine fused into
            # the exp (sc4/bi4 are constant along the softmax axis; softmax is
            # shift-invariant and post-BN values are unit-scale, so no max
            # subtraction is needed for fp32 range safety).
            wt_sb = {}
            for b in range(B_LOC):
                for kc in range(KC):
                    raw = wraw_sb[b, kc]
                    e = elup.tile([P, L], F32, tag="elu_e")
                    ssum = scr.tile([P, 1], F32, tag="scr")
                    nc.scalar.activation(
                        out=e[:], in_=raw[:], func=AF.Exp,
                        bias=bi4[kc][:], scale=sc4[kc][:], accum_out=ssum[:],
                    )
                    rs = scr.tile([P, 1], F32, tag="scr")
                    nc.vector.reciprocal(rs[:], ssum[:])
                    t = wtp.tile([P, L], F32, tag="wT")
                    nc.vector.tensor_scalar_mul(t[:].bitcast(F32R), e[:], rs[:])
                    wt_sb[b, kc] = t
            es_w4.close()
            es_z.close()

            # ---------------- Stage 5: out = w @ l, BN5 + ELU ----------------
            stats5 = [statp.tile([P, B_LOC, 6], F32, tag="st", name="st5") for _ in range(KC)]
            es_s5 = ExitStack()
            if True:
                lsp = es_s5.enter_context(tc.tile_pool(name="lstd", bufs=B_LOC * KC))
                outp = es_s5.enter_context(tc.tile_pool(name="out", bufs=B_LOC * KC))
                lstd_sb = {}
                for b in range(B_LOC):
                    for kc in range(KC):
                        lst = lsp.tile([P, E], F32, tag="lstd")
                        for mc in range(KC):
                            pst = psumT.tile([P, P], F32, tag="psT")
                            nc.tensor.transpose(
                                pst[:], l_sb[b, mc][:, kc * P : (kc + 1) * P], ident[:]
                            )
                            nc.vector.tensor_copy(
                                lst[:, mc * P : (mc + 1) * P].bitcast(F32R), pst[:]
                            )
                        lstd_sb[b, kc] = lst

                out_sb = {}
                for b in range(B_LOC):
                    for mc in range(KC):
                        ps = psum.tile([P, L], F32, tag="ps")
                        for kc in range(KC):
                            mmr(
                                ps[:],
                                lstd_sb[b, kc][:, mc * P : (mc + 1) * P],
                                wt_sb[b, kc][:],
                                start=(kc == 0),
                                stop=(kc == KC - 1),
                            )
                        ot = outp.tile([P, L], F32, tag="out")
                        nc.vector.tensor_copy(ot[:], ps[:])
                        nc.vector.bn_stats(out=stats5[mc][:, b, :], in_=ot[:])
                        out_sb[b, mc] = ot

                pack5 = packp.tile([P, KC, 2], F32, tag="g")
                stats_to_pack(stats5, pack5)
                g5 = allreduce(pack5, KC * 2)
                sc5, bi5 = affines(g5, _G5, _B5)
                for b in range(B_LOC):
                    for mc in range(KC):
                        elu_apply(out_sb[b, mc], sc5[mc], bi5[mc], round_out=False)
                        nc.sync.dma_start(
                            out=out_d[b, mc * P : (mc + 1) * P, :], in_=out_sb[b, mc][:]
                        )

                es_s5.close()
                es_l.close()

    _split_waits(nc)
    _CACHE["nc"] = nc
    return nc


def _pack_affine(vecs):
    cols = []
    for v in vecs:
        cols.append(np.ascontiguousarray(np.asarray(v, np.float32).reshape(KC, P).T))
    return np.ascontiguousarray(np.concatenate(cols, axis=1))


def kernel(m1, f, wq, wk, qb, kb, wb, g1, b1, g2, b2, g3, b3, g4, b4, g5, b5):
    m1 = np.asarray(m1, np.float32)
    nc = _build()
    # host-side zero pad along L so the kernel needs no memsets
    m1t = np.zeros((B, E, L + 2 * S), np.float32)
    m1t[:, :, S : S + L] = m1.transpose(0, 2, 1)
    f_h = np.ascontiguousarray(np.asarray(f, np.float32))
    wq_h = np.ascontiguousarray(np.asarray(wq, np.float32))
    wk_h = np.ascontiguousarray(np.asarray(wk, np.float32))
    qbt = np.ascontiguousarray(np.asarray(qb, np.float32).T)
    kbt = np.ascontiguousarray(np.asarray(kb, np.float32).T)
    wbt = np.ascontiguousarray(np.asarray(wb, np.float32).T)
    gpack = _pack_affine([g1, b1, g2, b2, g3, b3, g4, b4, g5, b5])

    shared = {
        "f": f_h, "wq": wq_h, "wk": wk_h,
        "qbt": qbt, "kbt": kbt, "wbt": wbt, "gpack": gpack,
    }
    in_maps = [
        {"m1t": np.ascontiguousarray(m1t[i * B_LOC : (i + 1) * B_LOC]), **shared}
        for i in range(N_CORES)
    ]
    trace = os.environ.get("KERNEL_TRACE") == "1"
    res = run_bass_kernel_spmd(nc, in_maps, list(range(N_CORES)), trace=trace)
    _CACHE["last_results"] = res

    out = np.empty((B, L, E), np.float32)
    for i in range(N_CORES):
        out[i * B_LOC : (i + 1) * B_LOC] = res.results[i]["outt"].transpose(0, 2, 1)
    return out

